# revision 15
# baseline (speedup 1.0000x reference)
"""GroupedEmbeddingBag kernel for 8 trn2 NeuronCores.

Table-parallel: core c handles table c (weights[c], values[c], offsets[c]).

Gather strategy: the vocab is split into 4 quartile shards of 25000 rows so
row indices fit dma_gather's int16 index format. Each shard's positions
(kept in original order) form a sub-list gathered by 6 big dma_gather
instructions (8704 rows each) — 24 SWDGE instructions total instead of 1600
small indirect DMAs, which removes the ~1us/instruction Pool (Q7
descriptor-generation) serialization that bounded earlier versions.

Pooling: TensorE matmuls against host-baked fp8 0/1 selection matrices
segment-sum each 128-row tile into PSUM "group" tiles (one group = 4
consecutive sub-list tiles = 512 sub-list slots, bag window W slots; sub-list
order preserves position order so windows stay small). Group results stream
to DRAM as bf16; the host maps (group, slot) -> bag and concatenates tables.
"""

import sys

sys.path.insert(0, "/opt/trn_rl_repo")

import numpy as np
import ml_dtypes

T, V, D, B = 8, 100000, 128, 4096
L = 204800
P = 128
NSH = 4                    # vocab shards
SHR = V // NSH             # 25000 rows per shard (int16-addressable)
CK = 8704                  # idxs per dma_gather (68 tiles)
CKT = CK // P              # 68 tiles per chunk
NCK = 6                    # chunks per shard; capacity 52224 >= max sub-list
GRP = 4                    # tiles per psum group
NTILESN = NSH * NCK * CKT  # 1632 tiles
NEPN = NTILESN // GRP      # 408 groups
OUT_RING_EP = 8            # groups per output DMA (408/8 = 51)

_compiled = {}


def _patch_drain(tile_mod, mybir):
    from concourse.vector_clock import ScopedClock

    def _patched(self, tick_clock, wait_clock):
        # this walrus build allows only ONE sync-wait on the tail Drain:
        # spread the rest over preceding nops, one wait each.
        NNOPS = 64
        nops = [self.nc.sync.nop(nofuse=True, hint=f"dw_{i}") for i in range(NNOPS)]
        drain_inst = self.nc.sync.drain()
        wait_clock.add_sem_waits(
            drain_inst.ins, ScopedClock({None: tick_clock.global_clock})
        )
        dsi = drain_inst.ins.sync_info
        waits = list(dsi.on_wait) if dsi else []
        if len(waits) > 1:
            del dsi.on_wait[1:]
            rest = waits[1:]
            assert len(rest) <= NNOPS, f"too many drain waits: {len(waits)}"
            for nop, w in zip(nops, rest):
                nsi = nop.ins.sync_info
                if nsi is None:
                    nop.ins.sync_info = mybir.SyncInfo(on_wait=[w], on_update=[])
                else:
                    nsi.on_wait.append(w)
        self.nc.all_engine_barrier()
        popped = self.nc._tile_sem_poison_stack.pop()
        assert popped is self._sem_poison
        self.nc.clear_and_free_semaphores(list(self.sems.allocated().values()))
        self.nc.all_engine_barrier()

    tile_mod.TileContext._drain_and_barrier = _patched


def _split_waits(nc, mybir, maxw=1):
    # this walrus build rejects >1 sync-wait on an instruction: hoist extra
    # waits onto same-engine nops spliced in directly before it.
    cnt = 0
    for fn in nc.m.functions:
        for blk in fn.blocks:
            new_insts = []
            for inst in blk.instructions:
                si = inst.sync_info
                if si is not None and len(si.on_wait) > maxw:
                    extra = list(si.on_wait[maxw:])
                    del si.on_wait[maxw:]
                    for w in extra:
                        nop = mybir.InstNoOp(
                            name=f"waitnop-{cnt}", engine=inst.engine, ins=[], outs=[]
                        )
                        cnt += 1
                        nop.sync_info = mybir.SyncInfo(on_wait=[w], on_update=[])
                        new_insts.append(nop)
                new_insts.append(inst)
            blk.instructions[:] = new_insts
    return cnt


def _build(W, for_sim=False, reps=1):
    import concourse.bass as bass
    import concourse.mybir as mybir
    import concourse.tile as tile
    from concourse.library_config import mlp

    if not hasattr(tile.TileContext, "_orig_drain_and_barrier"):
        tile.TileContext._orig_drain_and_barrier = tile.TileContext._drain_and_barrier
    if for_sim:
        tile.TileContext._drain_and_barrier = tile.TileContext._orig_drain_and_barrier
    else:
        _patch_drain(tile, mybir)

    nc = bass.Bass()
    wt = nc.declare_dram_parameter("wt", [V, D], mybir.dt.bfloat16, isOutput=False)
    idx = nc.declare_dram_parameter(
        "idx", [P, NSH * NCK * (CK // 16)], mybir.dt.int16, isOutput=False
    )
    sel = nc.declare_dram_parameter(
        "sel", [P, NTILESN * W], mybir.dt.float8e4, isOutput=False
    )
    oslots = nc.declare_dram_parameter(
        "oslots", [W, NEPN * D], mybir.dt.bfloat16, isOutput=True
    )

    with tile.TileContext(nc) as tc:
        with (
            tc.tile_pool(name="idxp", bufs=1) as idxp,
            tc.tile_pool(name="etp", bufs=3) as etp,
            tc.tile_pool(name="selp", bufs=3) as selp,
            tc.tile_pool(name="outp", bufs=2) as outp,
            tc.tile_pool(name="psum", bufs=8, space="PSUM") as psump,
        ):
            nc.gpsimd.load_library(mlp)
            idx_sb = idxp.tile([P, NSH * NCK * (CK // 16)], mybir.dt.int16)
            nc.sync.dma_start(out=idx_sb[:], in_=idx[:])
            out_ring = None
            psum_t = None
            for _rep in range(reps):
              for g in range(NSH):
                for c in range(NCK):
                    ch = g * NCK + c
                    et = etp.tile([P, CKT * D], mybir.dt.bfloat16, tag="et")
                    nc.gpsimd.dma_gather(
                        et[:].rearrange("p (n d) -> p n d", n=CKT),
                        wt[g * SHR:(g + 1) * SHR, :],
                        idx_sb[:, ch * (CK // 16):(ch + 1) * (CK // 16)],
                        CK,
                        CK,
                        D,
                        single_packet=False,
                    )
                    sel_sb = selp.tile([P, CKT * W], mybir.dt.float8e4, tag="sel")
                    nc.sync.dma_start(
                        out=sel_sb[:],
                        in_=sel[:, ch * CKT * W:(ch + 1) * CKT * W],
                    )
                    for tl in range(CKT):
                        gt = ch * CKT + tl
                        ph = tl % GRP
                        if ph == 0:
                            psum_t = psump.tile([W, D], mybir.dt.float32, tag="ps")
                        nc.tensor.matmul(
                            out=psum_t[:],
                            lhsT=sel_sb[:, tl * W:(tl + 1) * W],
                            rhs=et[:, tl * D:(tl + 1) * D],
                            start=(ph == 0),
                            stop=(ph == GRP - 1),
                        )
                        if ph == GRP - 1:
                            ge = gt // GRP
                            er = ge % OUT_RING_EP
                            if er == 0:
                                out_ring = outp.tile(
                                    [W, OUT_RING_EP * D], mybir.dt.bfloat16, tag="or"
                                )
                            nc.vector.tensor_copy(
                                out=out_ring[:, er * D:(er + 1) * D], in_=psum_t[:]
                            )
                            if er == OUT_RING_EP - 1:
                                e0 = ge - (OUT_RING_EP - 1)
                                nc.sync.dma_start(
                                    out=oslots[:, e0 * D:(e0 + OUT_RING_EP) * D],
                                    in_=out_ring[:],
                                )
    if not for_sim:
        _split_waits(nc, mybir)
    from concourse.library_overlay import lower_extended_insts
    lower_extended_insts(nc)
    return nc


def _host_prep(values_c, seg_c, W=None):
    """Build idx/sel layouts + group windows for one table."""
    cap = NCK * CK  # 52224 slots per shard
    sh = values_c // SHR
    streams = []
    for g in range(NSH):
        idxs_g = np.nonzero(sh == g)[0]
        assert len(idxs_g) <= cap, f"shard overflow: {len(idxs_g)} > {cap}"
        lid = (values_c[idxs_g] - g * SHR).astype(np.int16)
        segs = seg_c[idxs_g]
        streams.append((lid, segs))

    nslots = NSH * cap
    seg_stream = np.full(nslots, -1, dtype=np.int64)
    lid_stream = np.zeros(nslots, dtype=np.int16)
    for g, (lid, segs) in enumerate(streams):
        seg_stream[g * cap: g * cap + len(lid)] = segs
        lid_stream[g * cap: g * cap + len(lid)] = lid

    # group windows: group ge covers slots [512*ge, 512*(ge+1))
    segg = seg_stream.reshape(NEPN, GRP * P)
    valid = segg >= 0
    b_lo = np.where(valid.any(1), np.where(valid, segg, np.iinfo(np.int64).max).min(1), 0)
    b_hi = np.where(valid.any(1), np.where(valid, segg, -1).max(1), -1)
    S = np.maximum(b_hi - b_lo + 1, 0)
    return lid_stream, seg_stream, b_lo, S


def kernel(values, offsets, weights):
    from concourse.bass_utils import run_bass_kernel_spmd

    values = np.asarray(values)
    offsets = np.asarray(offsets)
    weights = np.asarray(weights, dtype=np.float32)

    pos = np.arange(L)
    seg = np.empty((T, L), dtype=np.int64)
    for c in range(T):
        seg[c] = np.searchsorted(offsets[c, 1:], pos, side="right")

    preps = [_host_prep(values[c], seg[c]) for c in range(T)]
    W = int(max(p[3].max() for p in preps))
    W = max(4, (W + 3) // 4 * 4)
    assert W <= 128, f"group bag-window {W} exceeds PSUM partition limit"

    in_maps = []
    for c in range(T):
        lid_stream, seg_stream, b_lo, S = preps[c]
        # idx param: per chunk ch, flat idxs wrapped [16, CK/16], tiled to 128
        idx = np.empty((P, NSH * NCK * (CK // 16)), dtype=np.int16)
        for ch in range(NSH * NCK):
            flat = lid_stream[ch * CK:(ch + 1) * CK]
            wrapped = flat.reshape(CK // 16, 16).T  # [16, CK/16]
            idx[:, ch * (CK // 16):(ch + 1) * (CK // 16)] = np.tile(wrapped, (8, 1))
        # sel: [P, NTILESN*W]; tile gt covers slots [128*gt, 128*(gt+1))
        seg_l = seg_stream.reshape(NTILESN, P).T  # [P, NTILESN]
        base = np.repeat(b_lo, GRP)               # [NTILESN]
        loc = seg_l - base[None, :]               # padded slots: negative
        selm = loc[:, :, None] == np.arange(W, dtype=np.int64)[None, None, :]
        sel = np.ascontiguousarray(
            selm.reshape(P, NTILESN * W).astype(ml_dtypes.float8_e4m3)
        )
        wt_bf = np.ascontiguousarray(weights[c].astype(ml_dtypes.bfloat16))
        in_maps.append({"wt": wt_bf, "idx": idx, "sel": sel})

    key = W
    if key not in _compiled:
        _compiled.clear()
        _compiled[key] = _build(W)
    nc = _compiled[key]

    global _last_inmaps
    _last_inmaps = in_maps
    res = run_bass_kernel_spmd(nc, in_maps, core_ids=list(range(T)))

    out = np.zeros((B, T * D), dtype=np.float32)
    for c in range(T):
        _, _, b_lo, S = preps[c]
        osl = res.results[c]["oslots"].astype(np.float32).reshape(W, NEPN, D)
        pooled = np.zeros((B, D), dtype=np.float32)
        for e in range(NEPN):
            n = int(S[e])
            if n:
                lo = int(b_lo[e])
                pooled[lo:lo + n] += osl[:n, e, :]
        out[:, c * D:(c + 1) * D] = pooled
    return out


if __name__ == "__main__":
    rng = np.random.default_rng(0)
    values = rng.integers(0, V, size=(T, L)).astype(np.int64)
    inner = np.sort(rng.integers(0, L, size=(T, B - 1)), axis=1)
    offsets = np.concatenate(
        [np.zeros((T, 1), np.int64), inner, np.full((T, 1), L, np.int64)], axis=1
    )
    weights = (rng.standard_normal((T, V, D)) * 0.01).astype(np.float32)
    out = kernel(values, offsets, weights)
    # numpy reference
    exp = np.zeros((B, T * D), dtype=np.float32)
    for c in range(T):
        pooled = np.zeros((B, D), np.float32)
        np.add.at(pooled, np.searchsorted(offsets[c, 1:], np.arange(L), side="right"), weights[c][values[c]])
        exp[:, c * D:(c + 1) * D] = pooled
    err = np.linalg.norm(out - exp) / np.linalg.norm(exp)
    print("self-check rel err:", err)


# revision 17
# speedup vs baseline: 2.2642x; 2.2642x over previous
"""GroupedEmbeddingBag kernel for 8 trn2 NeuronCores.

Table-parallel: core c handles table c (weights[c], values[c], offsets[c]).

Gather strategy: the vocab is split into 4 quartile shards of 25000 rows so
row indices fit dma_gather's int16 index format. Each shard's positions
(kept in original order) form a sub-list gathered by 6 big dma_gather
instructions (8704 rows each) — 24 SWDGE instructions total instead of 1600
small indirect DMAs, which removes the ~1us/instruction Pool (Q7
descriptor-generation) serialization that bounded earlier versions.

Pooling: TensorE matmuls against host-baked fp8 0/1 selection matrices
segment-sum each 128-row tile into PSUM "group" tiles (one group = 4
consecutive sub-list tiles = 512 sub-list slots, bag window W slots; sub-list
order preserves position order so windows stay small). Group results stream
to DRAM as bf16; the host maps (group, slot) -> bag and concatenates tables.
"""

import sys

sys.path.insert(0, "/opt/trn_rl_repo")

import numpy as np
import ml_dtypes

T, V, D, B = 8, 100000, 128, 4096
L = 204800
P = 128
NSH = 4                    # vocab shards
SHR = V // NSH             # 25000 rows per shard (int16-addressable)
CK = 8704                  # idxs per dma_gather (68 tiles)
CKT = CK // P              # 68 tiles per chunk
NCK = 6                    # chunks per shard; capacity 52224 >= max sub-list
GRP = 4                    # tiles per psum group
NTILESN = NSH * NCK * CKT  # 1632 tiles
NEPN = NTILESN // GRP      # 408 groups
OUT_RING_EP = 8            # groups per output DMA (408/8 = 51)

_compiled = {}


def _patch_drain(tile_mod, mybir):
    from concourse.vector_clock import ScopedClock

    def _patched(self, tick_clock, wait_clock):
        # this walrus build allows only ONE sync-wait on the tail Drain:
        # spread the rest over preceding nops, one wait each.
        NNOPS = 64
        nops = [self.nc.sync.nop(nofuse=True, hint=f"dw_{i}") for i in range(NNOPS)]
        drain_inst = self.nc.sync.drain()
        wait_clock.add_sem_waits(
            drain_inst.ins, ScopedClock({None: tick_clock.global_clock})
        )
        dsi = drain_inst.ins.sync_info
        waits = list(dsi.on_wait) if dsi else []
        if len(waits) > 1:
            del dsi.on_wait[1:]
            rest = waits[1:]
            assert len(rest) <= NNOPS, f"too many drain waits: {len(waits)}"
            for nop, w in zip(nops, rest):
                nsi = nop.ins.sync_info
                if nsi is None:
                    nop.ins.sync_info = mybir.SyncInfo(on_wait=[w], on_update=[])
                else:
                    nsi.on_wait.append(w)
        self.nc.all_engine_barrier()
        popped = self.nc._tile_sem_poison_stack.pop()
        assert popped is self._sem_poison
        self.nc.clear_and_free_semaphores(list(self.sems.allocated().values()))
        self.nc.all_engine_barrier()

    tile_mod.TileContext._drain_and_barrier = _patched


def _split_waits(nc, mybir, maxw=1):
    # this walrus build rejects >1 sync-wait on an instruction: hoist extra
    # waits onto same-engine nops spliced in directly before it.
    cnt = 0
    for fn in nc.m.functions:
        for blk in fn.blocks:
            new_insts = []
            for inst in blk.instructions:
                si = inst.sync_info
                if si is not None and len(si.on_wait) > maxw:
                    extra = list(si.on_wait[maxw:])
                    del si.on_wait[maxw:]
                    for w in extra:
                        nop = mybir.InstNoOp(
                            name=f"waitnop-{cnt}", engine=inst.engine, ins=[], outs=[]
                        )
                        cnt += 1
                        nop.sync_info = mybir.SyncInfo(on_wait=[w], on_update=[])
                        new_insts.append(nop)
                new_insts.append(inst)
            blk.instructions[:] = new_insts
    return cnt


def _build(W, for_sim=False, reps=1):
    import concourse.bass as bass
    import concourse.mybir as mybir
    import concourse.tile as tile
    from concourse.library_config import mlp

    if not hasattr(tile.TileContext, "_orig_drain_and_barrier"):
        tile.TileContext._orig_drain_and_barrier = tile.TileContext._drain_and_barrier
    if for_sim:
        tile.TileContext._drain_and_barrier = tile.TileContext._orig_drain_and_barrier
    else:
        _patch_drain(tile, mybir)

    nc = bass.Bass(num_swdge_queues=4, dynamic_dma_scratch_size=65536)
    wt = nc.declare_dram_parameter("wt", [V, D], mybir.dt.bfloat16, isOutput=False)
    idx = nc.declare_dram_parameter(
        "idx", [P, NSH * NCK * (CK // 16)], mybir.dt.int16, isOutput=False
    )
    sel = nc.declare_dram_parameter(
        "sel", [P, NTILESN * W], mybir.dt.float8e4, isOutput=False
    )
    oslots = nc.declare_dram_parameter(
        "oslots", [W, NEPN * D], mybir.dt.bfloat16, isOutput=True
    )

    with tile.TileContext(nc) as tc:
        with (
            tc.tile_pool(name="idxp", bufs=1) as idxp,
            tc.tile_pool(name="etp", bufs=3) as etp,
            tc.tile_pool(name="selp", bufs=3) as selp,
            tc.tile_pool(name="outp", bufs=2) as outp,
            tc.tile_pool(name="psum", bufs=8, space="PSUM") as psump,
        ):
            nc.gpsimd.load_library(mlp)
            idx_sb = idxp.tile([P, NSH * NCK * (CK // 16)], mybir.dt.int16)
            nc.sync.dma_start(out=idx_sb[:], in_=idx[:])
            out_ring = None
            psum_t = None
            for _rep in range(reps):
              for g in range(NSH):
                for c in range(NCK):
                    ch = g * NCK + c
                    et = etp.tile([P, CKT * D], mybir.dt.bfloat16, tag="et")
                    nc.gpsimd.dma_gather(
                        et[:].rearrange("p (n d) -> p n d", n=CKT),
                        wt[g * SHR:(g + 1) * SHR, :],
                        idx_sb[:, ch * (CK // 16):(ch + 1) * (CK // 16)],
                        CK,
                        CK,
                        D,
                        single_packet=False,
                        queue_num=ch % 4,
                    )
                    sel_sb = selp.tile([P, CKT * W], mybir.dt.float8e4, tag="sel")
                    nc.sync.dma_start(
                        out=sel_sb[:],
                        in_=sel[:, ch * CKT * W:(ch + 1) * CKT * W],
                    )
                    for tl in range(CKT):
                        gt = ch * CKT + tl
                        ph = tl % GRP
                        if ph == 0:
                            psum_t = psump.tile([W, D], mybir.dt.float32, tag="ps")
                        nc.tensor.matmul(
                            out=psum_t[:],
                            lhsT=sel_sb[:, tl * W:(tl + 1) * W],
                            rhs=et[:, tl * D:(tl + 1) * D],
                            start=(ph == 0),
                            stop=(ph == GRP - 1),
                        )
                        if ph == GRP - 1:
                            ge = gt // GRP
                            er = ge % OUT_RING_EP
                            if er == 0:
                                out_ring = outp.tile(
                                    [W, OUT_RING_EP * D], mybir.dt.bfloat16, tag="or"
                                )
                            nc.vector.tensor_copy(
                                out=out_ring[:, er * D:(er + 1) * D], in_=psum_t[:]
                            )
                            if er == OUT_RING_EP - 1:
                                e0 = ge - (OUT_RING_EP - 1)
                                nc.sync.dma_start(
                                    out=oslots[:, e0 * D:(e0 + OUT_RING_EP) * D],
                                    in_=out_ring[:],
                                )
    if not for_sim:
        _split_waits(nc, mybir)
    from concourse.library_overlay import lower_extended_insts
    lower_extended_insts(nc)
    return nc


def _host_prep(values_c, seg_c, W=None):
    """Build idx/sel layouts + group windows for one table."""
    cap = NCK * CK  # 52224 slots per shard
    sh = values_c // SHR
    streams = []
    for g in range(NSH):
        idxs_g = np.nonzero(sh == g)[0]
        assert len(idxs_g) <= cap, f"shard overflow: {len(idxs_g)} > {cap}"
        lid = (values_c[idxs_g] - g * SHR).astype(np.int16)
        segs = seg_c[idxs_g]
        streams.append((lid, segs))

    nslots = NSH * cap
    seg_stream = np.full(nslots, -1, dtype=np.int64)
    lid_stream = np.zeros(nslots, dtype=np.int16)
    for g, (lid, segs) in enumerate(streams):
        seg_stream[g * cap: g * cap + len(lid)] = segs
        lid_stream[g * cap: g * cap + len(lid)] = lid

    # group windows: group ge covers slots [512*ge, 512*(ge+1))
    segg = seg_stream.reshape(NEPN, GRP * P)
    valid = segg >= 0
    b_lo = np.where(valid.any(1), np.where(valid, segg, np.iinfo(np.int64).max).min(1), 0)
    b_hi = np.where(valid.any(1), np.where(valid, segg, -1).max(1), -1)
    S = np.maximum(b_hi - b_lo + 1, 0)
    return lid_stream, seg_stream, b_lo, S


def kernel(values, offsets, weights):
    from concourse.bass_utils import run_bass_kernel_spmd

    values = np.asarray(values)
    offsets = np.asarray(offsets)
    weights = np.asarray(weights, dtype=np.float32)

    pos = np.arange(L)
    seg = np.empty((T, L), dtype=np.int64)
    for c in range(T):
        seg[c] = np.searchsorted(offsets[c, 1:], pos, side="right")

    preps = [_host_prep(values[c], seg[c]) for c in range(T)]
    W = int(max(p[3].max() for p in preps))
    W = max(4, (W + 3) // 4 * 4)
    assert W <= 128, f"group bag-window {W} exceeds PSUM partition limit"

    in_maps = []
    for c in range(T):
        lid_stream, seg_stream, b_lo, S = preps[c]
        # idx param: per chunk ch, flat idxs wrapped [16, CK/16], tiled to 128
        idx = np.empty((P, NSH * NCK * (CK // 16)), dtype=np.int16)
        for ch in range(NSH * NCK):
            flat = lid_stream[ch * CK:(ch + 1) * CK]
            wrapped = flat.reshape(CK // 16, 16).T  # [16, CK/16]
            idx[:, ch * (CK // 16):(ch + 1) * (CK // 16)] = np.tile(wrapped, (8, 1))
        # sel: [P, NTILESN*W]; tile gt covers slots [128*gt, 128*(gt+1))
        seg_l = seg_stream.reshape(NTILESN, P).T  # [P, NTILESN]
        base = np.repeat(b_lo, GRP)               # [NTILESN]
        loc = seg_l - base[None, :]               # padded slots: negative
        selm = loc[:, :, None] == np.arange(W, dtype=np.int64)[None, None, :]
        sel = np.ascontiguousarray(
            selm.reshape(P, NTILESN * W).astype(ml_dtypes.float8_e4m3)
        )
        wt_bf = np.ascontiguousarray(weights[c].astype(ml_dtypes.bfloat16))
        in_maps.append({"wt": wt_bf, "idx": idx, "sel": sel})

    key = W
    if key not in _compiled:
        _compiled.clear()
        _compiled[key] = _build(W)
    nc = _compiled[key]

    global _last_inmaps
    _last_inmaps = in_maps
    res = run_bass_kernel_spmd(nc, in_maps, core_ids=list(range(T)))

    out = np.zeros((B, T * D), dtype=np.float32)
    for c in range(T):
        _, _, b_lo, S = preps[c]
        osl = res.results[c]["oslots"].astype(np.float32).reshape(W, NEPN, D)
        pooled = np.zeros((B, D), dtype=np.float32)
        for e in range(NEPN):
            n = int(S[e])
            if n:
                lo = int(b_lo[e])
                pooled[lo:lo + n] += osl[:n, e, :]
        out[:, c * D:(c + 1) * D] = pooled
    return out


if __name__ == "__main__":
    rng = np.random.default_rng(0)
    values = rng.integers(0, V, size=(T, L)).astype(np.int64)
    inner = np.sort(rng.integers(0, L, size=(T, B - 1)), axis=1)
    offsets = np.concatenate(
        [np.zeros((T, 1), np.int64), inner, np.full((T, 1), L, np.int64)], axis=1
    )
    weights = (rng.standard_normal((T, V, D)) * 0.01).astype(np.float32)
    out = kernel(values, offsets, weights)
    # numpy reference
    exp = np.zeros((B, T * D), dtype=np.float32)
    for c in range(T):
        pooled = np.zeros((B, D), np.float32)
        np.add.at(pooled, np.searchsorted(offsets[c, 1:], np.arange(L), side="right"), weights[c][values[c]])
        exp[:, c * D:(c + 1) * D] = pooled
    err = np.linalg.norm(out - exp) / np.linalg.norm(exp)
    print("self-check rel err:", err)


# revision 18
# speedup vs baseline: 3.8031x; 1.6797x over previous
"""GroupedEmbeddingBag kernel for 8 trn2 NeuronCores.

Table-parallel: core c handles table c (weights[c], values[c], offsets[c]).

Gather strategy: the vocab is split into 4 quartile shards of 25000 rows so
row indices fit dma_gather's int16 index format. Each shard's positions
(kept in original order) form a sub-list gathered by 6 big dma_gather
instructions (8704 rows each) — 24 SWDGE instructions total instead of 1600
small indirect DMAs, which removes the ~1us/instruction Pool (Q7
descriptor-generation) serialization that bounded earlier versions.

Pooling: TensorE matmuls against host-baked fp8 0/1 selection matrices
segment-sum each 128-row tile into PSUM "group" tiles (one group = 4
consecutive sub-list tiles = 512 sub-list slots, bag window W slots; sub-list
order preserves position order so windows stay small). Group results stream
to DRAM as bf16; the host maps (group, slot) -> bag and concatenates tables.
"""

import sys

sys.path.insert(0, "/opt/trn_rl_repo")

import numpy as np
import ml_dtypes

T, V, D, B = 8, 100000, 128, 4096
L = 204800
P = 128
NSH = 4                    # vocab shards
SHR = V // NSH             # 25000 rows per shard (int16-addressable)
CK = 8704                  # idxs per dma_gather (68 tiles)
CKT = CK // P              # 68 tiles per chunk
NCK = 6                    # chunks per shard; capacity 52224 >= max sub-list
GRP = 4                    # tiles per psum group
NTILESN = NSH * NCK * CKT  # 1632 tiles
NEPN = NTILESN // GRP      # 408 groups
OUT_RING_EP = 8            # groups per output DMA (408/8 = 51)

_compiled = {}


def _patch_drain(tile_mod, mybir):
    from concourse.vector_clock import ScopedClock

    def _patched(self, tick_clock, wait_clock):
        # this walrus build allows only ONE sync-wait on the tail Drain:
        # spread the rest over preceding nops, one wait each.
        NNOPS = 64
        nops = [self.nc.sync.nop(nofuse=True, hint=f"dw_{i}") for i in range(NNOPS)]
        drain_inst = self.nc.sync.drain()
        wait_clock.add_sem_waits(
            drain_inst.ins, ScopedClock({None: tick_clock.global_clock})
        )
        dsi = drain_inst.ins.sync_info
        waits = list(dsi.on_wait) if dsi else []
        if len(waits) > 1:
            del dsi.on_wait[1:]
            rest = waits[1:]
            assert len(rest) <= NNOPS, f"too many drain waits: {len(waits)}"
            for nop, w in zip(nops, rest):
                nsi = nop.ins.sync_info
                if nsi is None:
                    nop.ins.sync_info = mybir.SyncInfo(on_wait=[w], on_update=[])
                else:
                    nsi.on_wait.append(w)
        self.nc.all_engine_barrier()
        popped = self.nc._tile_sem_poison_stack.pop()
        assert popped is self._sem_poison
        self.nc.clear_and_free_semaphores(list(self.sems.allocated().values()))
        self.nc.all_engine_barrier()

    tile_mod.TileContext._drain_and_barrier = _patched


def _split_waits(nc, mybir, maxw=1):
    # this walrus build rejects >1 sync-wait on an instruction: hoist extra
    # waits onto same-engine nops spliced in directly before it.
    cnt = 0
    for fn in nc.m.functions:
        for blk in fn.blocks:
            new_insts = []
            for inst in blk.instructions:
                si = inst.sync_info
                if si is not None and len(si.on_wait) > maxw:
                    extra = list(si.on_wait[maxw:])
                    del si.on_wait[maxw:]
                    for w in extra:
                        nop = mybir.InstNoOp(
                            name=f"waitnop-{cnt}", engine=inst.engine, ins=[], outs=[]
                        )
                        cnt += 1
                        nop.sync_info = mybir.SyncInfo(on_wait=[w], on_update=[])
                        new_insts.append(nop)
                new_insts.append(inst)
            blk.instructions[:] = new_insts
    return cnt


def _build(W, for_sim=False, reps=1):
    import concourse.bass as bass
    import concourse.mybir as mybir
    import concourse.tile as tile
    from concourse.library_config import mlp

    if not hasattr(tile.TileContext, "_orig_drain_and_barrier"):
        tile.TileContext._orig_drain_and_barrier = tile.TileContext._drain_and_barrier
    if for_sim:
        tile.TileContext._drain_and_barrier = tile.TileContext._orig_drain_and_barrier
    else:
        _patch_drain(tile, mybir)

    nc = bass.Bass(num_swdge_queues=4, dynamic_dma_scratch_size=65536)
    wt = nc.declare_dram_parameter("wt", [V, D], mybir.dt.bfloat16, isOutput=False)
    idx = nc.declare_dram_parameter(
        "idx", [P, NSH * NCK * (CK // 16)], mybir.dt.int16, isOutput=False
    )
    sel = nc.declare_dram_parameter(
        "sel", [P, NTILESN * W], mybir.dt.float8e4, isOutput=False
    )
    oslots = nc.declare_dram_parameter(
        "oslots", [W, NEPN * D], mybir.dt.bfloat16, isOutput=True
    )

    with tile.TileContext(nc) as tc:
        with (
            tc.tile_pool(name="idxp", bufs=1) as idxp,
            tc.tile_pool(name="etp", bufs=5) as etp,
            tc.tile_pool(name="selp", bufs=3) as selp,
            tc.tile_pool(name="outp", bufs=2) as outp,
            tc.tile_pool(name="psum", bufs=8, space="PSUM") as psump,
        ):
            nc.gpsimd.load_library(mlp)
            idx_sb = idxp.tile([P, NSH * NCK * (CK // 16)], mybir.dt.int16)
            nc.sync.dma_start(out=idx_sb[:], in_=idx[:])
            out_ring = None
            psum_t = None
            for _rep in range(reps):
              for g in range(NSH):
                for c in range(NCK):
                    ch = g * NCK + c
                    et = etp.tile([P, CKT * D], mybir.dt.bfloat16, tag="et")
                    nc.gpsimd.dma_gather(
                        et[:].rearrange("p (n d) -> p n d", n=CKT),
                        wt[g * SHR:(g + 1) * SHR, :],
                        idx_sb[:, ch * (CK // 16):(ch + 1) * (CK // 16)],
                        CK,
                        CK,
                        D,
                        single_packet=False,
                        queue_num=ch % 4,
                    )
                    sel_sb = selp.tile([P, CKT * W], mybir.dt.float8e4, tag="sel")
                    nc.sync.dma_start(
                        out=sel_sb[:],
                        in_=sel[:, ch * CKT * W:(ch + 1) * CKT * W],
                    )
                    for tl in range(CKT):
                        gt = ch * CKT + tl
                        ph = tl % GRP
                        if ph == 0:
                            psum_t = psump.tile([W, D], mybir.dt.float32, tag="ps")
                        nc.tensor.matmul(
                            out=psum_t[:],
                            lhsT=sel_sb[:, tl * W:(tl + 1) * W],
                            rhs=et[:, tl * D:(tl + 1) * D],
                            start=(ph == 0),
                            stop=(ph == GRP - 1),
                        )
                        if ph == GRP - 1:
                            ge = gt // GRP
                            er = ge % OUT_RING_EP
                            if er == 0:
                                out_ring = outp.tile(
                                    [W, OUT_RING_EP * D], mybir.dt.bfloat16, tag="or"
                                )
                            nc.vector.tensor_copy(
                                out=out_ring[:, er * D:(er + 1) * D], in_=psum_t[:]
                            )
                            if er == OUT_RING_EP - 1:
                                e0 = ge - (OUT_RING_EP - 1)
                                nc.sync.dma_start(
                                    out=oslots[:, e0 * D:(e0 + OUT_RING_EP) * D],
                                    in_=out_ring[:],
                                )
    if not for_sim:
        _split_waits(nc, mybir)
    from concourse.library_overlay import lower_extended_insts
    lower_extended_insts(nc)
    return nc


def _host_prep(values_c, seg_c, W=None):
    """Build idx/sel layouts + group windows for one table."""
    cap = NCK * CK  # 52224 slots per shard
    sh = values_c // SHR
    streams = []
    for g in range(NSH):
        idxs_g = np.nonzero(sh == g)[0]
        assert len(idxs_g) <= cap, f"shard overflow: {len(idxs_g)} > {cap}"
        lid = (values_c[idxs_g] - g * SHR).astype(np.int16)
        segs = seg_c[idxs_g]
        streams.append((lid, segs))

    nslots = NSH * cap
    seg_stream = np.full(nslots, -1, dtype=np.int64)
    lid_stream = np.zeros(nslots, dtype=np.int16)
    for g, (lid, segs) in enumerate(streams):
        seg_stream[g * cap: g * cap + len(lid)] = segs
        lid_stream[g * cap: g * cap + len(lid)] = lid

    # group windows: group ge covers slots [512*ge, 512*(ge+1))
    segg = seg_stream.reshape(NEPN, GRP * P)
    valid = segg >= 0
    b_lo = np.where(valid.any(1), np.where(valid, segg, np.iinfo(np.int64).max).min(1), 0)
    b_hi = np.where(valid.any(1), np.where(valid, segg, -1).max(1), -1)
    S = np.maximum(b_hi - b_lo + 1, 0)
    return lid_stream, seg_stream, b_lo, S


def kernel(values, offsets, weights):
    from concourse.bass_utils import run_bass_kernel_spmd

    values = np.asarray(values)
    offsets = np.asarray(offsets)
    weights = np.asarray(weights, dtype=np.float32)

    pos = np.arange(L)
    seg = np.empty((T, L), dtype=np.int64)
    for c in range(T):
        seg[c] = np.searchsorted(offsets[c, 1:], pos, side="right")

    preps = [_host_prep(values[c], seg[c]) for c in range(T)]
    W = int(max(p[3].max() for p in preps))
    W = max(4, (W + 3) // 4 * 4)
    assert W <= 128, f"group bag-window {W} exceeds PSUM partition limit"

    in_maps = []
    for c in range(T):
        lid_stream, seg_stream, b_lo, S = preps[c]
        # idx param: per chunk ch, flat idxs wrapped [16, CK/16], tiled to 128
        idx = np.empty((P, NSH * NCK * (CK // 16)), dtype=np.int16)
        for ch in range(NSH * NCK):
            flat = lid_stream[ch * CK:(ch + 1) * CK]
            wrapped = flat.reshape(CK // 16, 16).T  # [16, CK/16]
            idx[:, ch * (CK // 16):(ch + 1) * (CK // 16)] = np.tile(wrapped, (8, 1))
        # sel: [P, NTILESN*W]; tile gt covers slots [128*gt, 128*(gt+1))
        seg_l = seg_stream.reshape(NTILESN, P).T  # [P, NTILESN]
        base = np.repeat(b_lo, GRP)               # [NTILESN]
        loc = seg_l - base[None, :]               # padded slots: negative
        selm = loc[:, :, None] == np.arange(W, dtype=np.int64)[None, None, :]
        sel = np.ascontiguousarray(
            selm.reshape(P, NTILESN * W).astype(ml_dtypes.float8_e4m3)
        )
        wt_bf = np.ascontiguousarray(weights[c].astype(ml_dtypes.bfloat16))
        in_maps.append({"wt": wt_bf, "idx": idx, "sel": sel})

    key = W
    if key not in _compiled:
        _compiled.clear()
        _compiled[key] = _build(W)
    nc = _compiled[key]

    global _last_inmaps
    _last_inmaps = in_maps
    res = run_bass_kernel_spmd(nc, in_maps, core_ids=list(range(T)))

    out = np.zeros((B, T * D), dtype=np.float32)
    for c in range(T):
        _, _, b_lo, S = preps[c]
        osl = res.results[c]["oslots"].astype(np.float32).reshape(W, NEPN, D)
        pooled = np.zeros((B, D), dtype=np.float32)
        for e in range(NEPN):
            n = int(S[e])
            if n:
                lo = int(b_lo[e])
                pooled[lo:lo + n] += osl[:n, e, :]
        out[:, c * D:(c + 1) * D] = pooled
    return out


if __name__ == "__main__":
    rng = np.random.default_rng(0)
    values = rng.integers(0, V, size=(T, L)).astype(np.int64)
    inner = np.sort(rng.integers(0, L, size=(T, B - 1)), axis=1)
    offsets = np.concatenate(
        [np.zeros((T, 1), np.int64), inner, np.full((T, 1), L, np.int64)], axis=1
    )
    weights = (rng.standard_normal((T, V, D)) * 0.01).astype(np.float32)
    out = kernel(values, offsets, weights)
    # numpy reference
    exp = np.zeros((B, T * D), dtype=np.float32)
    for c in range(T):
        pooled = np.zeros((B, D), np.float32)
        np.add.at(pooled, np.searchsorted(offsets[c, 1:], np.arange(L), side="right"), weights[c][values[c]])
        exp[:, c * D:(c + 1) * D] = pooled
    err = np.linalg.norm(out - exp) / np.linalg.norm(exp)
    print("self-check rel err:", err)


# revision 19
# speedup vs baseline: 13.1003x; 3.4446x over previous
"""GroupedEmbeddingBag kernel for 8 trn2 NeuronCores.

Table-parallel: core c handles table c (weights[c], values[c], offsets[c]).

Gather strategy: the vocab is split into 4 quartile shards of 25000 rows so
row indices fit dma_gather's int16 index format. Each shard's positions
(kept in original order) form a sub-list gathered by 6 big dma_gather
instructions (8704 rows each) — 24 SWDGE instructions total instead of 1600
small indirect DMAs, which removes the ~1us/instruction Pool (Q7
descriptor-generation) serialization that bounded earlier versions.

Pooling: TensorE matmuls against host-baked fp8 0/1 selection matrices
segment-sum each 128-row tile into PSUM "group" tiles (one group = 4
consecutive sub-list tiles = 512 sub-list slots, bag window W slots; sub-list
order preserves position order so windows stay small). Group results stream
to DRAM as bf16; the host maps (group, slot) -> bag and concatenates tables.
"""

import sys

sys.path.insert(0, "/opt/trn_rl_repo")

import numpy as np
import ml_dtypes

T, V, D, B = 8, 100000, 128, 4096
L = 204800
P = 128
NSH = 4                    # vocab shards
SHR = V // NSH             # 25000 rows per shard (int16-addressable)
CK = 8704                  # idxs per dma_gather (68 tiles)
CKT = CK // P              # 68 tiles per chunk
NCK = 6                    # chunks per shard; capacity 52224 >= max sub-list
GRP = 4                    # tiles per psum group
NTILESN = NSH * NCK * CKT  # 1632 tiles
NEPN = NTILESN // GRP      # 408 groups
OUT_RING_EP = 8            # groups per output DMA (408/8 = 51)

_compiled = {}


def _patch_drain(tile_mod, mybir):
    from concourse.vector_clock import ScopedClock

    def _patched(self, tick_clock, wait_clock):
        # this walrus build allows only ONE sync-wait on the tail Drain:
        # spread the rest over preceding nops, one wait each.
        NNOPS = 64
        nops = [self.nc.sync.nop(nofuse=True, hint=f"dw_{i}") for i in range(NNOPS)]
        drain_inst = self.nc.sync.drain()
        wait_clock.add_sem_waits(
            drain_inst.ins, ScopedClock({None: tick_clock.global_clock})
        )
        dsi = drain_inst.ins.sync_info
        waits = list(dsi.on_wait) if dsi else []
        if len(waits) > 1:
            del dsi.on_wait[1:]
            rest = waits[1:]
            assert len(rest) <= NNOPS, f"too many drain waits: {len(waits)}"
            for nop, w in zip(nops, rest):
                nsi = nop.ins.sync_info
                if nsi is None:
                    nop.ins.sync_info = mybir.SyncInfo(on_wait=[w], on_update=[])
                else:
                    nsi.on_wait.append(w)
        self.nc.all_engine_barrier()
        popped = self.nc._tile_sem_poison_stack.pop()
        assert popped is self._sem_poison
        self.nc.clear_and_free_semaphores(list(self.sems.allocated().values()))
        self.nc.all_engine_barrier()

    tile_mod.TileContext._drain_and_barrier = _patched


def _split_waits(nc, mybir, maxw=1):
    # this walrus build rejects >1 sync-wait on an instruction: hoist extra
    # waits onto same-engine nops spliced in directly before it.
    cnt = 0
    for fn in nc.m.functions:
        for blk in fn.blocks:
            new_insts = []
            for inst in blk.instructions:
                si = inst.sync_info
                if si is not None and len(si.on_wait) > maxw:
                    extra = list(si.on_wait[maxw:])
                    del si.on_wait[maxw:]
                    for w in extra:
                        nop = mybir.InstNoOp(
                            name=f"waitnop-{cnt}", engine=inst.engine, ins=[], outs=[]
                        )
                        cnt += 1
                        nop.sync_info = mybir.SyncInfo(on_wait=[w], on_update=[])
                        new_insts.append(nop)
                new_insts.append(inst)
            blk.instructions[:] = new_insts
    return cnt


def _build(W, for_sim=False, reps=1):
    import concourse.bass as bass
    import concourse.mybir as mybir
    import concourse.tile as tile
    from concourse.library_config import mlp

    if not hasattr(tile.TileContext, "_orig_drain_and_barrier"):
        tile.TileContext._orig_drain_and_barrier = tile.TileContext._drain_and_barrier
    if for_sim:
        tile.TileContext._drain_and_barrier = tile.TileContext._orig_drain_and_barrier
    else:
        _patch_drain(tile, mybir)

    nc = bass.Bass(num_swdge_queues=4, dynamic_dma_scratch_size=65536)
    wt = nc.declare_dram_parameter("wt", [V, D], mybir.dt.bfloat16, isOutput=False)
    idx = nc.declare_dram_parameter(
        "idx", [P, NSH * NCK * (CK // 16)], mybir.dt.int16, isOutput=False
    )
    sel = nc.declare_dram_parameter(
        "sel", [P, NTILESN * W], mybir.dt.float8e4, isOutput=False
    )
    oslots = nc.declare_dram_parameter(
        "oslots", [W, NEPN * D], mybir.dt.bfloat16, isOutput=True
    )

    with tile.TileContext(nc) as tc:
        with (
            tc.tile_pool(name="idxp", bufs=1) as idxp,
            tc.tile_pool(name="etp", bufs=6) as etp,
            tc.tile_pool(name="selp", bufs=4) as selp,
            tc.tile_pool(name="outp", bufs=2) as outp,
            tc.tile_pool(name="psum", bufs=8, space="PSUM") as psump,
        ):
            nc.gpsimd.load_library(mlp)
            idx_sb = idxp.tile([P, NSH * NCK * (CK // 16)], mybir.dt.int16)
            nc.sync.dma_start(out=idx_sb[:], in_=idx[:])
            out_ring = None
            psum_t = None
            for _rep in range(reps):
              for g in range(NSH):
                for c in range(NCK):
                    ch = g * NCK + c
                    et = etp.tile([P, CKT * D], mybir.dt.bfloat16, tag="et")
                    nc.gpsimd.dma_gather(
                        et[:].rearrange("p (n d) -> p n d", n=CKT),
                        wt[g * SHR:(g + 1) * SHR, :],
                        idx_sb[:, ch * (CK // 16):(ch + 1) * (CK // 16)],
                        CK,
                        CK,
                        D,
                        single_packet=False,
                        queue_num=ch % 4,
                    )
                    sel_sb = selp.tile([P, CKT * W], mybir.dt.float8e4, tag="sel")
                    nc.sync.dma_start(
                        out=sel_sb[:],
                        in_=sel[:, ch * CKT * W:(ch + 1) * CKT * W],
                    )
                    for tl in range(CKT):
                        gt = ch * CKT + tl
                        ph = tl % GRP
                        if ph == 0:
                            psum_t = psump.tile([W, D], mybir.dt.float32, tag="ps")
                        nc.tensor.matmul(
                            out=psum_t[:],
                            lhsT=sel_sb[:, tl * W:(tl + 1) * W],
                            rhs=et[:, tl * D:(tl + 1) * D],
                            start=(ph == 0),
                            stop=(ph == GRP - 1),
                        )
                        if ph == GRP - 1:
                            ge = gt // GRP
                            er = ge % OUT_RING_EP
                            if er == 0:
                                out_ring = outp.tile(
                                    [W, OUT_RING_EP * D], mybir.dt.bfloat16, tag="or"
                                )
                            nc.vector.tensor_copy(
                                out=out_ring[:, er * D:(er + 1) * D], in_=psum_t[:]
                            )
                            if er == OUT_RING_EP - 1:
                                e0 = ge - (OUT_RING_EP - 1)
                                nc.sync.dma_start(
                                    out=oslots[:, e0 * D:(e0 + OUT_RING_EP) * D],
                                    in_=out_ring[:],
                                )
    if not for_sim:
        _split_waits(nc, mybir)
    from concourse.library_overlay import lower_extended_insts
    lower_extended_insts(nc)
    return nc


def _host_prep(values_c, seg_c, W=None):
    """Build idx/sel layouts + group windows for one table."""
    cap = NCK * CK  # 52224 slots per shard
    sh = values_c // SHR
    streams = []
    for g in range(NSH):
        idxs_g = np.nonzero(sh == g)[0]
        assert len(idxs_g) <= cap, f"shard overflow: {len(idxs_g)} > {cap}"
        lid = (values_c[idxs_g] - g * SHR).astype(np.int16)
        segs = seg_c[idxs_g]
        streams.append((lid, segs))

    nslots = NSH * cap
    seg_stream = np.full(nslots, -1, dtype=np.int64)
    lid_stream = np.zeros(nslots, dtype=np.int16)
    for g, (lid, segs) in enumerate(streams):
        seg_stream[g * cap: g * cap + len(lid)] = segs
        lid_stream[g * cap: g * cap + len(lid)] = lid

    # group windows: group ge covers slots [512*ge, 512*(ge+1))
    segg = seg_stream.reshape(NEPN, GRP * P)
    valid = segg >= 0
    b_lo = np.where(valid.any(1), np.where(valid, segg, np.iinfo(np.int64).max).min(1), 0)
    b_hi = np.where(valid.any(1), np.where(valid, segg, -1).max(1), -1)
    S = np.maximum(b_hi - b_lo + 1, 0)
    return lid_stream, seg_stream, b_lo, S


def kernel(values, offsets, weights):
    from concourse.bass_utils import run_bass_kernel_spmd

    values = np.asarray(values)
    offsets = np.asarray(offsets)
    weights = np.asarray(weights, dtype=np.float32)

    pos = np.arange(L)
    seg = np.empty((T, L), dtype=np.int64)
    for c in range(T):
        seg[c] = np.searchsorted(offsets[c, 1:], pos, side="right")

    preps = [_host_prep(values[c], seg[c]) for c in range(T)]
    W = int(max(p[3].max() for p in preps))
    W = max(4, (W + 3) // 4 * 4)
    assert W <= 128, f"group bag-window {W} exceeds PSUM partition limit"

    in_maps = []
    for c in range(T):
        lid_stream, seg_stream, b_lo, S = preps[c]
        # idx param: per chunk ch, flat idxs wrapped [16, CK/16], tiled to 128
        idx = np.empty((P, NSH * NCK * (CK // 16)), dtype=np.int16)
        for ch in range(NSH * NCK):
            flat = lid_stream[ch * CK:(ch + 1) * CK]
            wrapped = flat.reshape(CK // 16, 16).T  # [16, CK/16]
            idx[:, ch * (CK // 16):(ch + 1) * (CK // 16)] = np.tile(wrapped, (8, 1))
        # sel: [P, NTILESN*W]; tile gt covers slots [128*gt, 128*(gt+1))
        seg_l = seg_stream.reshape(NTILESN, P).T  # [P, NTILESN]
        base = np.repeat(b_lo, GRP)               # [NTILESN]
        loc = seg_l - base[None, :]               # padded slots: negative
        selm = loc[:, :, None] == np.arange(W, dtype=np.int64)[None, None, :]
        sel = np.ascontiguousarray(
            selm.reshape(P, NTILESN * W).astype(ml_dtypes.float8_e4m3)
        )
        wt_bf = np.ascontiguousarray(weights[c].astype(ml_dtypes.bfloat16))
        in_maps.append({"wt": wt_bf, "idx": idx, "sel": sel})

    key = W
    if key not in _compiled:
        _compiled.clear()
        _compiled[key] = _build(W)
    nc = _compiled[key]

    global _last_inmaps
    _last_inmaps = in_maps
    res = run_bass_kernel_spmd(nc, in_maps, core_ids=list(range(T)))

    out = np.zeros((B, T * D), dtype=np.float32)
    for c in range(T):
        _, _, b_lo, S = preps[c]
        osl = res.results[c]["oslots"].astype(np.float32).reshape(W, NEPN, D)
        pooled = np.zeros((B, D), dtype=np.float32)
        for e in range(NEPN):
            n = int(S[e])
            if n:
                lo = int(b_lo[e])
                pooled[lo:lo + n] += osl[:n, e, :]
        out[:, c * D:(c + 1) * D] = pooled
    return out


if __name__ == "__main__":
    rng = np.random.default_rng(0)
    values = rng.integers(0, V, size=(T, L)).astype(np.int64)
    inner = np.sort(rng.integers(0, L, size=(T, B - 1)), axis=1)
    offsets = np.concatenate(
        [np.zeros((T, 1), np.int64), inner, np.full((T, 1), L, np.int64)], axis=1
    )
    weights = (rng.standard_normal((T, V, D)) * 0.01).astype(np.float32)
    out = kernel(values, offsets, weights)
    # numpy reference
    exp = np.zeros((B, T * D), dtype=np.float32)
    for c in range(T):
        pooled = np.zeros((B, D), np.float32)
        np.add.at(pooled, np.searchsorted(offsets[c, 1:], np.arange(L), side="right"), weights[c][values[c]])
        exp[:, c * D:(c + 1) * D] = pooled
    err = np.linalg.norm(out - exp) / np.linalg.norm(exp)
    print("self-check rel err:", err)
